# revision 1
# baseline (speedup 1.0000x reference)
"""AutoCorrelation (Autoformer-style) forward on 8 Trainium2 NeuronCores.

kernel(**inputs) takes FULL unsharded inputs, returns the FULL (B, L, D) output.

Sharding: 32 (batch, head) pairs split 4-per-core (cores 0-3 batch 0, cores 4-7
batch 1). The ENTIRE pipeline runs on device per core: Q/K/V projections
(fp16 operands, fp32 PSUM), circular autocorrelation via Q@K^T accumulated
into rotated PSUM windows, a 3-stage shear (indirect_copy per 16-partition
group + permutation matmuls between stages) and a ones-matmul partition
reduction to get corr[d], top-8 delay selection (max_with_indices), softmax,
weighted circular gather of V (indirect_copy), and the output projection
(row-sharded; partials summed on host with bo).

Hardcoded shapes: B=2, L=4096, D=1024, H=16, Dk=64, top_k=8.
Self-contained: reads nothing from /root/problem.
"""

import math
import sys

import numpy as np

if "/opt/trn_rl_repo" not in sys.path:
    sys.path.insert(0, "/opt/trn_rl_repo")

B = 2
L = 4096
D_MODEL = 1024
NHEAD = 16
DK = D_MODEL // NHEAD  # 64
TOP_K = min(max(1, int(math.log(L + 1))), L)  # 8
N_CORES = 8
HPC = 4  # heads per core
COLS = HPC * DK  # 256 projection columns per core


# ---------------------------------------------------------------------------
# host fallback (numerically exact, slow) — used if the device path fails
# ---------------------------------------------------------------------------
def _tail_host(Q, K, V, Wo, bo):
    Qf = np.fft.rfft(Q, axis=2)
    Kf = np.fft.rfft(K, axis=2)
    corr = np.fft.irfft(Qf * np.conj(Kf), n=L, axis=2)
    corr_mean = corr.mean(axis=-1).astype(np.float32)

    idx = np.argsort(-corr_mean, axis=-1, kind="stable")[..., :TOP_K]
    w = np.take_along_axis(corr_mean, idx, axis=-1)
    w = np.exp(w - w.max(axis=-1, keepdims=True))
    w = w / w.sum(axis=-1, keepdims=True)

    out = np.zeros((B, NHEAD, L, DK), dtype=np.float32)
    ar = np.arange(L)
    for b in range(B):
        for h in range(NHEAD):
            acc = np.zeros((L, DK), dtype=np.float32)
            for t in range(TOP_K):
                acc += w[b, h, t] * V[b, h][(ar + int(idx[b, h, t])) % L]
            out[b, h] = acc

    out = out.transpose(0, 2, 1, 3).reshape(B * L, D_MODEL)
    return (out @ Wo + bo).reshape(B, L, D_MODEL).astype(np.float32)


def _forward_host(query, key, value, Wq, bq, Wk, bk, Wv, bv, Wo, bo):
    def proj(x, W, b):
        p = (x.reshape(B * L, D_MODEL) @ W + b).astype(np.float32)
        return p.reshape(B, L, NHEAD, DK).transpose(0, 2, 1, 3)

    return _tail_host(proj(query, Wq, bq), proj(key, Wk, bk), proj(value, Wv, bv), Wo, bo)


# ---------------------------------------------------------------------------
# device kernel
# ---------------------------------------------------------------------------
_NC_CACHE = {}


def _const_tables():
    """Constant tables for the on-device shear/gather.

    Shear: corr[d] = sum_p racc[p, (p - d) % L]; per-row shift p = 16g + 2w1
    + w0 applied via three per-16-group indirect_copy stages with row
    permutations (matmuls) between: A shifts 16g (+ index reversal), then
    rows regroup by w1 (pi1), B shifts 2*w1, regroup by w0 (pi2), C shifts w0.
    """

    def ic_table(lists):
        t = np.zeros((128, 256), np.uint16)
        for g in range(8):
            t[16 * g : 16 * (g + 1), :] = lists[g].reshape(256, 16).T
        return t

    taba = ic_table([(16 * g - np.arange(L)) % L for g in range(8)])
    tabb = ic_table([(np.arange(L) - 2 * g) % L for g in range(8)])
    tabc = ic_table([(np.arange(L) - g // 4) % L for g in range(8)])

    gtbl = np.zeros((128, 256), np.uint16)
    for p in range(128):
        gtbl[p] = 16 * np.arange(256) + (p % 16)

    pi1 = np.zeros(128, np.int64)
    for p in range(128):
        g, w = p // 16, p % 16
        pi1[p] = 16 * (w // 2) + 2 * g + (w % 2)
    pi2 = np.array([64 * (p % 2) + p // 2 for p in range(128)])
    pm1 = np.zeros((128, 128), np.float16)
    pm1[np.arange(128), pi1] = 1.0
    pm2 = np.zeros((128, 128), np.float16)
    pm2[np.arange(128), pi2] = 1.0
    oe = ((np.arange(128) % 2 == 0).astype(np.float16)).reshape(128, 1)
    oo = ((np.arange(128) % 2 == 1).astype(np.float16)).reshape(128, 1)
    return taba, tabb, tabc, gtbl, pm1, pm2, oe, oo


def _fixed_filename(fn, fname="ac_kernel.py"):
    import types

    def fix(code):
        consts = tuple(
            fix(c) if isinstance(c, types.CodeType) else c for c in code.co_consts
        )
        return code.replace(co_consts=consts, co_filename=fname)

    g = types.FunctionType(
        fix(fn.__code__), fn.__globals__, fn.__name__, fn.__defaults__, fn.__closure__
    )
    return g


def _build_nc_impl(stop=5):
    import concourse.bacc as bacc
    import concourse.mybir as mybir
    from concourse.ap import AP
    from concourse.tile import TileContext

    f32 = mybir.dt.float32
    f16 = mybir.dt.float16
    u16 = mybir.dt.uint16
    u32 = mybir.dt.uint32

    nc = bacc.Bacc(None, target_bir_lowering=False, dynamic_dma_scratch_size=2048, disable_frame_to_traceback=True, name="ac")

    ins = {}
    for nm in ("xq", "xk", "xv"):
        ins[nm] = nc.declare_dram_parameter(nm, [D_MODEL, L], f16, isOutput=False)
    for nm in ("wq", "wk", "wv"):
        ins[nm] = nc.declare_dram_parameter(nm, [D_MODEL, COLS], f16, isOutput=False)
    ins["wo"] = nc.declare_dram_parameter("wo", [COLS, D_MODEL], f16, isOutput=False)
    for nm in ("taba", "tabb", "tabc", "gtbl"):
        ins[nm] = nc.declare_dram_parameter(nm, [128, 256], u16, isOutput=False)
    for nm in ("pm1", "pm2"):
        ins[nm] = nc.declare_dram_parameter(nm, [128, 128], f16, isOutput=False)
    for nm in ("oe", "oo"):
        ins[nm] = nc.declare_dram_parameter(nm, [128, 1], f16, isOutput=False)

    out_d = nc.declare_dram_parameter("out", [L, D_MODEL], f16, isOutput=True)
    ti_d = nc.declare_dram_parameter("ti", [HPC, 8], u32, isOutput=True)
    w8_d = nc.declare_dram_parameter("w8", [HPC, 8], f32, isOutput=True)

    KT = D_MODEL // 128  # 8 contraction chunks
    NW = L // 512  # 8 t-windows

    with TileContext(nc) as tc:
        with (
            tc.tile_pool(name="wp", bufs=1) as wp,
            tc.tile_pool(name="xs", bufs=2) as xs,
            tc.tile_pool(name="qkv", bufs=1) as qkv,
            tc.tile_pool(name="shp", bufs=1) as shp,
            tc.tile_pool(name="gp", bufs=1) as gp,
            tc.tile_pool(name="sm", bufs=1) as sm,
            tc.tile_pool(name="oev", bufs=2) as oevp,
            tc.tile_pool(name="pp", bufs=1, space="PSUM") as pp,
            tc.tile_pool(name="dr", bufs=1, space="DRAM") as dr,
        ):
            P8 = pp.tile([128, L], f32, tag="P8")

            # ---- constants
            tabs = {}
            for nm in ("taba", "tabb", "tabc", "gtbl"):
                t = sm.tile([128, 256], u16, tag=nm, name=nm)
                nc.sync.dma_start(out=t[:, :], in_=ins[nm][:, :])
                tabs[nm] = t
            pms = {}
            for nm in ("pm1", "pm2"):
                t = sm.tile([128, 128], f16, tag=nm, name=nm)
                nc.sync.dma_start(out=t[:, :], in_=ins[nm][:, :])
                pms[nm] = t
            ones16 = sm.tile([128, 1], f16, tag="ones16")
            nc.vector.memset(ones16[:, :], 1.0)
            oe_t = sm.tile([128, 1], f16, tag="oe", name="oe")
            nc.sync.dma_start(out=oe_t[:, :], in_=ins["oe"][:, :])
            oo_t = sm.tile([128, 1], f16, tag="oo", name="oo")
            nc.sync.dma_start(out=oo_t[:, :], in_=ins["oo"][:, :])
            gtblf_t = sm.tile([128, 256], f32, tag="gtblf")
            nc.vector.tensor_copy(gtblf_t[:, :], tabs["gtbl"][:, :])

            # ---- weights
            wt = {}
            for nm in ("wq", "wk", "wv"):
                t = wp.tile([128, KT * COLS], f16, tag=nm, name=nm)
                for kc in range(KT):
                    nc.sync.dma_start(
                        out=t[:, kc * COLS : (kc + 1) * COLS],
                        in_=ins[nm][kc * 128 : (kc + 1) * 128, :],
                    )
                wt[nm] = t
            wo_t = []
            for q in range(2):
                t = wp.tile([128, D_MODEL], f16, tag=f"wo{q}", name=f"wo{q}")
                nc.sync.dma_start(out=t[:, :], in_=ins["wo"][q * 128 : (q + 1) * 128, :])
                wo_t.append(t)

            # ---- projections -> QT/KT/VT pair tiles [128ch x L] f16
            proj = {"q": [], "k": [], "v": []}
            for key_ in ("q", "k", "v"):
                for q in range(2):
                    proj[key_].append(
                        qkv.tile([128, L], f16, tag=f"{key_}t{q}", name=f"{key_}t{q}")
                    )
            pswin = 0
            for key_, xnm, wnm in (("q", "xq", "wq"), ("k", "xk", "wk"), ("v", "xv", "wv")):
                for n in range(NW):
                    xst = xs.tile([128, KT * 512], f16, tag="xst")
                    for kc in range(KT):
                        nc.sync.dma_start(
                            out=xst[:, kc * 512 : (kc + 1) * 512],
                            in_=ins[xnm][kc * 128 : (kc + 1) * 128, n * 512 : (n + 1) * 512],
                        )
                    for m in range(2):
                        win = (pswin % 4) * 512
                        pswin += 1
                        for kc in range(KT):
                            nc.tensor.matmul(
                                P8[:, win : win + 512],
                                wt[wnm][:, kc * COLS + m * 128 : kc * COLS + (m + 1) * 128],
                                xst[:, kc * 512 : (kc + 1) * 512],
                                start=(kc == 0),
                                stop=(kc == KT - 1),
                                skip_group_check=True,
                            )
                        nc.vector.tensor_copy(
                            proj[key_][m][:, n * 512 : (n + 1) * 512], P8[:, win : win + 512]
                        )

            def zero_fill(outputs):
                zt = oevp.tile([128, 512], f16, tag="ot", name="zf")
                nc.vector.memset(zt[:, :], 0.0)
                if "out" in outputs:
                    for j in range(32):
                        for nn in range(2):
                            nc.sync.dma_start(
                                out=out_d[128 * j : 128 * (j + 1), 512 * nn : 512 * (nn + 1)],
                                in_=zt[:, :],
                            )
                if "ti" in outputs:
                    zi = sm.tile([1, 8], u32, tag="zi", name="zi")
                    nc.vector.memset(zi[:, :], 0)
                    zw = sm.tile([1, 8], f32, tag="zw", name="zw")
                    nc.vector.memset(zw[:, :], 0.0)
                    for h in range(HPC):
                        nc.sync.dma_start(out=ti_d[h : h + 1, :], in_=zi[:, :])
                        nc.sync.dma_start(out=w8_d[h : h + 1, :], in_=zw[:, :])


            if stop <= 1:
                zero_fill({"out", "corr", "ti"})
                # ---- tail per head: perm1 -> B -> perm2 -> C -> ones reduction
            tif_t, w8_t = [], []

            vw = [None, None]

            def _combine_pair(q):
                hA, hB = 2 * q, 2 * q + 1
                db = dr.tile([4, 8], f32, tag="db", name=f"db{q}")
                nc.sync.dma_start(out=db[0:1, :], in_=tif_t[hA][0:1, :])
                nc.sync.dma_start(out=db[1:2, :], in_=tif_t[hB][0:1, :])
                nc.sync.dma_start(out=db[2:3, :], in_=w8_t[hA][0:1, :])
                nc.sync.dma_start(out=db[3:4, :], in_=w8_t[hB][0:1, :])
                dcol = sm.tile([128, 8], f32, tag=f"dcol{q}", name=f"dcol{q}")
                wcol = sm.tile([128, 8], f32, tag=f"wcol{q}", name=f"wcol{q}")
                nc.sync.dma_start(out=dcol[0:64, :], in_=AP(db.tensor, 0, [[0, 64], [1, 8]]))
                nc.sync.dma_start(out=dcol[64:128, :], in_=AP(db.tensor, 8, [[0, 64], [1, 8]]))
                nc.sync.dma_start(out=wcol[0:64, :], in_=AP(db.tensor, 16, [[0, 64], [1, 8]]))
                nc.sync.dma_start(out=wcol[64:128, :], in_=AP(db.tensor, 24, [[0, 64], [1, 8]]))

                # two alternating accumulators halve the fmadd RAW chain
                vwa = gp.tile([128, L], f16, tag=f"vwa{q}", bufs=1, name=f"vwa{q}")
                nc.vector.memset(vwa[:, :], 0.0)
                for k in range(TOP_K):
                    idxf = gp.tile([128, 256], f32, tag="idxf", bufs=1)
                    nc.vector.tensor_scalar(
                        idxf[:, :], gtblf_t[:, :], dcol[:, k : k + 1], None,
                        mybir.AluOpType.add,
                    )
                    ge = gp.tile([128, 256], f32, tag="ge", bufs=1)
                    nc.vector.tensor_scalar(
                        ge[:, :], idxf[:, :], 4096.0, None, mybir.AluOpType.is_ge
                    )
                    nc.vector.scalar_tensor_tensor(
                        idxf[:, :], ge[:, :], -4096.0, idxf[:, :],
                        mybir.AluOpType.mult, mybir.AluOpType.add,
                    )
                    idxk = gp.tile([128, 256], u16, tag="idxk", bufs=1)
                    nc.vector.tensor_copy(idxk[:, :], idxf[:, :])
                    gk = gp.tile([128, L], f16, tag="gk", bufs=2)
                    for cc in range(4):
                        nc.gpsimd.indirect_copy(
                            gk[:, 1024 * cc : 1024 * (cc + 1)],
                            proj["v"][q][:, :],
                            idxk[:, 64 * cc : 64 * (cc + 1)],
                            True,
                        )
                    nc.vector.scalar_tensor_tensor(
                        vwa[:, :], gk[:, :], wcol[:, k : k + 1], vwa[:, :],
                        mybir.AluOpType.mult, mybir.AluOpType.add,
                    )
                vw[q] = vwa

            def _racc_head(h):
                pair, half = h // 2, h % 2
                rows = slice(64 * half, 64 * half + 64)
                qt, kt = proj["q"][pair], proj["k"][pair]
                for i in range(32):
                    lhs = qt[rows, 128 * i : 128 * (i + 1)]
                    r = (-128 * i) % 512
                    for bblk in range(NW):
                        e0 = (512 * bblk - 128 * i) % L
                        pieces = (
                            [(e0, 512, 0)]
                            if r == 0
                            else [(e0, 512 - r, 0), ((e0 + 512 - r) % L, r, 512 - r)]
                        )
                        for pe, plen, soff in pieces:
                            nc.tensor.matmul(
                                P8[:, pe : pe + plen],
                                lhs,
                                kt[rows, 512 * bblk + soff : 512 * bblk + soff + plen],
                                start=(i == 0),
                                stop=(i == 31),
                                skip_group_check=True,
                            )
                # evict fp32 PSUM -> fp16 SBUF, then shear stage A (shift 16g + reversal)
                a0 = shp.tile([128, L], f16, tag="a0", bufs=2)
                for j in range(NW):
                    nc.vector.tensor_copy(
                        a0[:, 512 * j : 512 * (j + 1)], P8[:, 512 * j : 512 * (j + 1)]
                    )
                a1 = shp.tile([128, L], f16, tag="a1", bufs=4)
                for cc in range(4):
                    nc.gpsimd.indirect_copy(
                        a1[:, 1024 * cc : 1024 * (cc + 1)],
                        a0[:, :],
                        tabs["taba"][:, 64 * cc : 64 * (cc + 1)],
                        True,
                    )
                return a1

            def _tail_head(h, a1):
                b0 = shp.tile([128, L], f16, tag="sE", bufs=2, name="b0")
                for j in range(NW):
                    win = 512 * j
                    nc.tensor.matmul(
                        P8[:, win : win + 512], pms["pm1"][:, :],
                        a1[:, win : win + 512], start=True, stop=True,
                        skip_group_check=True,
                    )
                    nc.vector.tensor_copy(b0[:, win : win + 512], P8[:, win : win + 512])
                b1 = shp.tile([128, L], f16, tag="sF", bufs=2, name="b1")
                for cc in range(4):
                    nc.gpsimd.indirect_copy(
                        b1[:, 1024 * cc : 1024 * (cc + 1)], b0[:, :],
                        tabs["tabb"][:, 64 * cc : 64 * (cc + 1)], True,
                    )
                # final reduction folds stage C: residual shift w0 = p' mod 2, so
                # corr[d] = sum_even B1[p',d] + sum_odd B1[p',(d-1)%L]
                for j in range(NW):
                    win = 512 * j
                    nc.tensor.matmul(
                        P8[0:1, win : win + 512], oe_t[:, :],
                        b1[:, win : win + 512], start=True, stop=False,
                        skip_group_check=True,
                    )
                    if j == 0:
                        nc.tensor.matmul(
                            P8[0:1, 0:1], oo_t[:, :], b1[:, L - 1 : L],
                            start=False, stop=False, skip_group_check=True,
                        )
                        nc.tensor.matmul(
                            P8[0:1, 1:512], oo_t[:, :], b1[:, 0:511],
                            start=False, stop=True, skip_group_check=True,
                        )
                    else:
                        nc.tensor.matmul(
                            P8[0:1, win : win + 512], oo_t[:, :],
                            b1[:, win - 1 : win + 511],
                            start=False, stop=True, skip_group_check=True,
                        )
                co = sm.tile([1, L], f32, tag="corr")
                for j in range(NW):
                    nc.vector.tensor_copy(
                        co[:, 512 * j : 512 * (j + 1)], P8[0:1, 512 * j : 512 * (j + 1)]
                    )

                tv = sm.tile([1, 8], f32, tag=f"tv{h}", name=f"tv{h}")
                ti = sm.tile([1, 8], u32, tag=f"ti{h}", name=f"ti{h}")
                nc.vector.max_with_indices(tv[:, :], ti[:, :], co[:, :])
                nc.sync.dma_start(out=ti_d[h : h + 1, :], in_=ti[:, :])
                tif = sm.tile([1, 8], f32, tag=f"tif{h}", name=f"tif{h}")
                nc.vector.tensor_copy(tif[:, :], ti[:, :])
                tif_t.append(tif)

                negmax = sm.tile([1, 1], f32, tag=f"nm{h}", name=f"nm{h}")
                nc.vector.tensor_scalar_mul(negmax[:, :], tv[:, 0:1], -1.0)
                e8 = sm.tile([1, 8], f32, tag=f"e8{h}", name=f"e8{h}")
                nc.scalar.activation(
                    e8[:, :], tv[:, :], mybir.ActivationFunctionType.Exp,
                    bias=negmax[:, 0:1], scale=1.0,
                )
                ssum = sm.tile([1, 1], f32, tag=f"ss{h}", name=f"ss{h}")
                nc.vector.tensor_reduce(
                    ssum[:, :], e8[:, :], mybir.AxisListType.X, mybir.AluOpType.add
                )
                rs = sm.tile([1, 1], f32, tag=f"rs{h}", name=f"rs{h}")
                nc.vector.reciprocal(rs[:, :], ssum[:, :])
                w8 = sm.tile([1, 8], f32, tag=f"w8{h}", name=f"w8{h}")
                nc.vector.tensor_scalar_mul(w8[:, :], e8[:, :], rs[:, 0:1])
                w8_t.append(w8)
                nc.sync.dma_start(out=w8_d[h : h + 1, :], in_=w8[:, :])

            if stop >= 2:
                for qb in range(2):
                    a1p = [_racc_head(hh) for hh in (2 * qb, 2 * qb + 1)]
                    if stop >= 3:
                        for ii, hh in enumerate((2 * qb, 2 * qb + 1)):
                            _tail_head(hh, a1p[ii])
                        if stop >= 4:
                            _combine_pair(qb)
            if stop == 2:
                zero_fill({"out", "corr", "ti"})
            if stop == 3:
                zero_fill({"out"})


            if stop == 3:
                zero_fill({"out"})
            if stop == 4:
                zero_fill({"out"})
            if stop >= 5:
                # ---- output projection: out[t,:] = sum_h vw_h[t,:] @ wo_h
                pswin = 0
                for j in range(32):
                    for nn in range(2):
                        w1 = (pswin % 4) * 512
                        w2 = w1 + 2048
                        pswin += 1
                        # same lhsT partition base within each accumulation group
                        for half, win in ((0, w1), (1, w2)):
                            rows = slice(64 * half, 64 * half + 64)
                            for pair in range(2):
                                nc.tensor.matmul(
                                    P8[:, win : win + 512],
                                    vw[pair][rows, 128 * j : 128 * (j + 1)],
                                    wo_t[pair][rows, 512 * nn : 512 * (nn + 1)],
                                    start=(pair == 0),
                                    stop=(pair == 1),
                                    skip_group_check=True,
                                )
                        ot = oevp.tile([128, 512], f16, tag="ot")
                        nc.vector.tensor_copy(ot[:, :], P8[:, w2 : w2 + 512])
                        nc.vector.tensor_add(ot[:, :], P8[:, w1 : w1 + 512], ot[:, :])
                        nc.sync.dma_start(
                            out=out_d[128 * j : 128 * (j + 1), 512 * nn : 512 * (nn + 1)],
                            in_=ot[:, :],
                        )

    # scrub path-dependent debug info so the NEFF cache key is directory-independent
    import concourse.mybir as mybir
    for alloc in nc.m.functions[0].allocations:
        if isinstance(alloc, mybir.MemoryLocationSet):
            for ml in alloc.memorylocations:
                try:
                    ml.ant_debug = None
                except Exception:
                    pass
    nc.finalize()
    return nc


_build_nc = _fixed_filename(_build_nc_impl)


def _build_into_impl(box, stop):
    box["nc"] = _build_nc(stop)


_build_into = _fixed_filename(_build_into_impl)


def _get_nc(stop=5):
    if stop not in _NC_CACHE:
        # build in a fresh thread: instruction debug tracebacks then capture a
        # deterministic stack (thread bootstrap only), keeping the serialized
        # BIR — and hence the NEFF compile-cache key — caller-independent
        import threading

        box = {}
        t = threading.Thread(target=_build_into, args=(box, stop), name="acb")
        t.start()
        t.join()
        _NC_CACHE[stop] = box["nc"]
    return _NC_CACHE[stop]


def _core_inputs(query, key, value, Wq, Wk, Wv, Wo):
    taba, tabb, tabc, gtbl, pm1, pm2, oe, oo = _const_tables()
    f16 = np.float16
    xT = {}
    for b in range(B):
        xT[("q", b)] = np.ascontiguousarray(query[b].T, dtype=f16)
        xT[("k", b)] = np.ascontiguousarray(key[b].T, dtype=f16)
        xT[("v", b)] = np.ascontiguousarray(value[b].T, dtype=f16)
    in_maps = []
    for c in range(N_CORES):
        b = c // 4
        h0 = (c % 4) * HPC
        cols = slice(h0 * DK, h0 * DK + COLS)
        in_maps.append({
            "xq": xT[("q", b)],
            "xk": xT[("k", b)],
            "xv": xT[("v", b)],
            "wq": Wq[:, cols].astype(f16),
            "wk": (Wk[:, cols] / DK).astype(f16),  # fold corr mean(1/Dk) into K
            "wv": Wv[:, cols].astype(f16),
            "wo": Wo[h0 * DK : h0 * DK + COLS, :].astype(f16),
            "taba": taba, "tabb": tabb, "tabc": tabc, "gtbl": gtbl,
            "pm1": pm1, "pm2": pm2, "oe": oe, "oo": oo,
        })
    return in_maps


class _Runner:
    """Cached AOT-compiled SPMD dispatch with device-side donated zero buffers."""

    def __init__(self, nc):
        import jax
        import jax.numpy as jnp
        from jax.sharding import Mesh, NamedSharding, PartitionSpec
        from jax.experimental.shard_map import shard_map
        import concourse.mybir as mybir
        from concourse.bass2jax import (
            _bass_exec_p, install_neuronx_cc_hook, partition_id_tensor,
        )

        install_neuronx_cc_hook()
        self.jax = jax
        pname = nc.partition_id_tensor.name if nc.partition_id_tensor else None
        in_names, out_names, out_avals = [], [], []
        for alloc in nc.m.functions[0].allocations:
            if not isinstance(alloc, mybir.MemoryLocationSet):
                continue
            name = alloc.memorylocations[0].name
            if alloc.kind == "ExternalInput":
                if name != pname:
                    in_names.append(name)
            elif alloc.kind == "ExternalOutput":
                out_names.append(name)
                out_avals.append(
                    jax.core.ShapedArray(tuple(alloc.tensor_shape), mybir.dt.np(alloc.dtype))
                )
        self.in_names, self.out_names, self.out_avals = in_names, out_names, out_avals
        n_params, n_outs = len(in_names), len(out_avals)
        all_names = in_names + out_names + ([pname] if pname else [])

        def _body(*a):
            operands = list(a)
            if pname is not None:
                operands.append(partition_id_tensor())
            return tuple(
                _bass_exec_p.bind(
                    *operands, out_avals=tuple(out_avals), in_names=tuple(all_names),
                    out_names=tuple(out_names), lowering_input_output_aliases=(),
                    sim_require_finite=True, sim_require_nnan=True, nc=nc,
                )
            )

        devices = jax.devices()[:N_CORES]
        self.mesh = Mesh(np.asarray(devices), ("core",))
        spec = PartitionSpec("core")
        self.sharding = NamedSharding(self.mesh, spec)
        donate = tuple(range(n_params, n_params + n_outs))
        fn = jax.jit(
            shard_map(
                _body, mesh=self.mesh, in_specs=(spec,) * (n_params + n_outs),
                out_specs=(spec,) * n_outs, check_rep=False,
            ),
            donate_argnums=donate, keep_unused=True,
        )
        gin = [
            jax.ShapeDtypeStruct((N_CORES * s[0], *s[1:]), d)
            for s, d in self._global_shapes(nc)
        ]
        gzero = [
            jax.ShapeDtypeStruct((N_CORES * a.shape[0], *a.shape[1:]), a.dtype)
            for a in out_avals
        ]
        self.compiled = fn.lower(*gin, *gzero).compile()
        zshapes = [((N_CORES * a.shape[0], *a.shape[1:]), a.dtype) for a in out_avals]
        self.zeros_fn = jax.jit(
            lambda: tuple(jnp.zeros(s, d) for s, d in zshapes),
            out_shardings=(self.sharding,) * n_outs,
        )

    def _global_shapes(self, nc):
        import concourse.mybir as mybir

        shapes = []
        for alloc in nc.m.functions[0].allocations:
            if not isinstance(alloc, mybir.MemoryLocationSet):
                continue
            name = alloc.memorylocations[0].name
            if alloc.kind == "ExternalInput" and name in self.in_names:
                shapes.append((tuple(alloc.tensor_shape), mybir.dt.np(alloc.dtype)))
        return shapes

    def stage(self, in_maps):
        cat = [
            np.concatenate([np.asarray(m[n]) for m in in_maps], axis=0)
            for n in self.in_names
        ]
        bufs = [self.jax.device_put(a, self.sharding) for a in cat]
        self.jax.block_until_ready(bufs)
        return bufs

    def make_zeros(self):
        z = self.zeros_fn()
        self.jax.block_until_ready(z)
        return z

    def exec_staged(self, bufs, zeros=None):
        if zeros is None:
            zeros = self.make_zeros()
        outs = self.compiled(*bufs, *zeros)
        self.jax.block_until_ready(outs)
        return outs

    def run(self, in_maps):
        outs = self.exec_staged(self.stage(in_maps))
        res = []
        for c in range(N_CORES):
            m = {}
            for i, nm in enumerate(self.out_names):
                s = self.out_avals[i].shape
                m[nm] = np.asarray(outs[i][c * s[0] : (c + 1) * s[0]])
            res.append(m)
        return res


class _Res:
    def __init__(self, results):
        self.results = results


def _get_runner(stop=5):
    key = ("runner", stop)
    if key not in _NC_CACHE:
        _NC_CACHE[key] = _Runner(_get_nc(stop))
    return _NC_CACHE[key]


def _forward_device(query, key, value, Wq, bq, Wk, bk, Wv, bv, Wo, bo, spmd_kwargs=None, stop=5):
    runner = _get_runner(stop)
    in_maps = _core_inputs(query, key, value, Wq, Wk, Wv, Wo)
    results = runner.run(in_maps)

    out = np.zeros((B, L, D_MODEL), dtype=np.float32)
    for c in range(N_CORES):
        out[c // 4] += np.asarray(results[c]["out"], dtype=np.float32)
    out += bo.astype(np.float32)
    return out, _Res(results)


def kernel(**inputs):
    inputs = {k: np.asarray(v, dtype=np.float32) for k, v in inputs.items()}
    if any(np.any(inputs[k]) for k in ("bq", "bk", "bv")):
        return _forward_host(**inputs)
    try:
        out, _ = _forward_device(**inputs)
        return out
    except Exception:
        import traceback

        traceback.print_exc()
        return _forward_host(**inputs)



# revision 23
# speedup vs baseline: 206.3355x; 206.3355x over previous
"""AutoCorrelation (Autoformer-style) forward on 8 Trainium2 NeuronCores.

kernel(**inputs) takes FULL unsharded inputs, returns the FULL (B, L, D) output.

Sharding: 32 (batch, head) pairs split 4-per-core (cores 0-3 batch 0, cores 4-7
batch 1). Per core, the whole pipeline runs on device:
  1. Q/K/V projections (fp16 operands, fp32 PSUM accumulate).
  2. Circular cross-correlation racc[p,e] = sum_i K[128i+p].Q[(e+128i)%L] via
     matmuls with full 128-partition contraction (two 128-t chunks stacked on
     partitions: K in a folded layout, Q in a 128-shifted stacked layout).
  3. Shear: racc rows go to DRAM (pitch 4224, 128-col wrap tail) and come back
     through a diagonal static AP (partition stride 4225) so that column d of
     the reloaded tile holds racc[p,(p+d)%L]; a ones-vector matmul then gives
     corr[d] directly. No indirect copies or permutation matmuls.
  4. top-8 delays (max_with_indices) + softmax on-device.
  5. Weighted circular gather of V: V goes to DRAM duplicated ([128, 2L]);
     indirect_dma_start with per-partition offsets p*2L + d_k pulls rolled
     rows with one fat 8KB descriptor per partition; vector ops accumulate
     the softmax-weighted sum.
  6. Output projection (row-sharded over heads; partials summed on host + bo).

Hardcoded shapes: B=2, L=4096, D=1024, H=16, Dk=64, top_k=8.
Self-contained: reads nothing from /root/problem.
"""

import math
import sys

import numpy as np

if "/opt/trn_rl_repo" not in sys.path:
    sys.path.insert(0, "/opt/trn_rl_repo")

B = 2
L = 4096
D_MODEL = 1024
NHEAD = 16
DK = D_MODEL // NHEAD  # 64
TOP_K = min(max(1, int(math.log(L + 1))), L)  # 8
N_CORES = 8
HPC = 4  # heads per core
COLS = HPC * DK  # 256 projection columns per core
REXT = L + 128  # 4224 sheared-scratch row pitch
VEXT = 2 * L  # 8192 duplicated V row pitch


# ---------------------------------------------------------------------------
# host fallback (numerically exact, slow) — used if the device path fails
# ---------------------------------------------------------------------------
def _tail_host(Q, K, V, Wo, bo):
    Qf = np.fft.rfft(Q, axis=2)
    Kf = np.fft.rfft(K, axis=2)
    corr = np.fft.irfft(Qf * np.conj(Kf), n=L, axis=2)
    corr_mean = corr.mean(axis=-1).astype(np.float32)

    idx = np.argsort(-corr_mean, axis=-1, kind="stable")[..., :TOP_K]
    w = np.take_along_axis(corr_mean, idx, axis=-1)
    w = np.exp(w - w.max(axis=-1, keepdims=True))
    w = w / w.sum(axis=-1, keepdims=True)

    out = np.zeros((B, NHEAD, L, DK), dtype=np.float32)
    ar = np.arange(L)
    for b in range(B):
        for h in range(NHEAD):
            acc = np.zeros((L, DK), dtype=np.float32)
            for t in range(TOP_K):
                acc += w[b, h, t] * V[b, h][(ar + int(idx[b, h, t])) % L]
            out[b, h] = acc

    out = out.transpose(0, 2, 1, 3).reshape(B * L, D_MODEL)
    return (out @ Wo + bo).reshape(B, L, D_MODEL).astype(np.float32)


def _forward_host(query, key, value, Wq, bq, Wk, bk, Wv, bv, Wo, bo):
    def proj(x, W, b):
        p = (x.reshape(B * L, D_MODEL) @ W + b).astype(np.float32)
        return p.reshape(B, L, NHEAD, DK).transpose(0, 2, 1, 3)

    return _tail_host(proj(query, Wq, bq), proj(key, Wk, bk), proj(value, Wv, bv), Wo, bo)


# ---------------------------------------------------------------------------
# device kernel
# ---------------------------------------------------------------------------
_NC_CACHE = {}


def _fixed_filename(fn, fname="ac_kernel.py"):
    import types

    def fix(code):
        consts = tuple(
            fix(c) if isinstance(c, types.CodeType) else c for c in code.co_consts
        )
        return code.replace(co_consts=consts, co_filename=fname)

    g = types.FunctionType(
        fix(fn.__code__), fn.__globals__, fn.__name__, fn.__defaults__, fn.__closure__
    )
    return g


def _build_nc_impl():
    import concourse.bacc as bacc
    import concourse.mybir as mybir
    from concourse import bass
    from concourse.ap import AP
    from concourse.tile import TileContext

    f32 = mybir.dt.float32
    f16 = mybir.dt.float16
    u32 = mybir.dt.uint32

    nc = bacc.Bacc(None, target_bir_lowering=False, dynamic_dma_scratch_size=2048,
                   disable_frame_to_traceback=True, name="ac")

    ins = {}
    for nm in ("xq", "xk", "xv"):
        ins[nm] = nc.declare_dram_parameter(nm, [D_MODEL, L], f16, isOutput=False)
    for nm in ("wq", "wk", "wv"):
        ins[nm] = nc.declare_dram_parameter(nm, [D_MODEL, COLS], f16, isOutput=False)
    ins["wo"] = nc.declare_dram_parameter("wo", [COLS, D_MODEL], f16, isOutput=False)
    ins["masks"] = nc.declare_dram_parameter("masks", [1, 256], f32, isOutput=False)
    ins["prow"] = nc.declare_dram_parameter("prow", [1, 128], f32, isOutput=False)

    out_d = nc.declare_dram_parameter("out", [L, D_MODEL], f16, isOutput=True)

    KT = D_MODEL // 128  # 8 contraction chunks for the projections
    NW = L // 512  # 8 512-wide windows

    with TileContext(nc) as tc:
        with (
            tc.tile_pool(name="wp", bufs=1) as wp,
            tc.tile_pool(name="xs", bufs=2) as xs,
            tc.tile_pool(name="qkv", bufs=1) as qkv,
            tc.tile_pool(name="work", bufs=1) as wk_,
            tc.tile_pool(name="sm", bufs=1) as sm,
            tc.tile_pool(name="pp", bufs=1, space="PSUM") as pp,
            tc.tile_pool(name="dr", bufs=1, space="DRAM") as dr,
        ):
            # ---- constants / weights
            masks_t = sm.tile([1, 256], f32, tag="masks", name="masks")
            nc.sync.dma_start(out=masks_t[:, :], in_=ins["masks"][:, :])
            prow_t = sm.tile([1, 128], f32, tag="prow", name="prow")
            nc.sync.dma_start(out=prow_t[:, :], in_=ins["prow"][:, :])
            ones16 = sm.tile([128, 1], f16, tag="ones16")
            nc.vector.memset(ones16[:, :], 1.0)
            ones8 = sm.tile([1, 8], f32, tag="ones8")
            nc.vector.memset(ones8[:, :], 1.0)

            wt = {}

            def load_w(nm):
                t = wp.tile([128, KT * COLS], f16, tag=nm, name=nm)
                for half in range(2):
                    nc.sync.dma_start(
                        out=t[:, half * 4 * COLS : (half + 1) * 4 * COLS],
                        in_=AP(ins[nm], half * 4 * 128 * COLS,
                               [[COLS, 128], [128 * COLS, 4], [1, COLS]]),
                    )
                wt[nm] = t

            load_w("wk")
            wo_t = []

            # ---- per-head persistent tiles
            # KTfold[h]: [128,2048]  rows (r,c) -> K[256j+128r+p, c] at col 128j+p
            # QT2[h]:    [128,4352]  rows 0:64 = QT[c,t]; rows 64:128 = QT[c,(t+128)%L]
            #            cols 4096:4352 duplicate cols 0:256
            ktf = [qkv.tile([128, 2048], f16, tag=f"ktf{h}", name=f"ktf{h}") for h in range(HPC)]
            qt2 = [qkv.tile([128, 4352], f16, tag=f"qt2{h}", name=f"qt2{h}") for h in range(HPC)]
            vt = [qkv.tile([128, L], f16, tag=f"vt{q}", name=f"vt{q}") for q in range(2)]

            # DRAM scratch
            rx = [[dr.tile([128, 1152], f16, tag=f"rx{h}_{sc}", name=f"rx{h}_{sc}")
                   for sc in range(4)] for h in range(HPC)]
            vext = [dr.tile([128, VEXT], f16, tag=f"vext{q}", name=f"vext{q}") for q in range(2)]

            # ---- projections: K, Q, V order
            def proj_pass(xnm, wnm, kind):
                for n in range(NW):
                    xst = xs.tile([128, KT * 512], f16, tag="xst")
                    for qtr in range(4):
                        nc.sync.dma_start(
                            out=xst[:, qtr * 2 * 512 : (qtr + 1) * 2 * 512],
                            in_=AP(ins[xnm], 512 * n + qtr * 2 * 128 * L,
                                   [[L, 128], [128 * L, 2], [1, 512]]),
                        )
                    for m in range(2):  # pair index
                        P = pp.tile([128, 512], f32, tag="mm", bufs=6)
                        for kc in range(KT):
                            nc.tensor.matmul(
                                P[:, :],
                                wt[wnm][:, kc * COLS + m * 128 : kc * COLS + (m + 1) * 128],
                                xst[:, kc * 512 : (kc + 1) * 512],
                                start=(kc == 0),
                                stop=(kc == KT - 1),
                                skip_group_check=True,
                            )
                        for hh in range(2):  # head within pair
                            h = 2 * m + hh
                            rows = slice(64 * hh, 64 * hh + 64)
                            if kind == "k":
                                # folded layout: r=0 <- psum cols {0:128,256:384},
                                # r=1 <- {128:256,384:512}; merged 3D-AP copies
                                src0 = AP(P.tensor, 64 * hh * 512,
                                          [[512, 64], [256, 2], [1, 128]])
                                src1 = AP(P.tensor, 64 * hh * 512 + 128,
                                          [[512, 64], [256, 2], [1, 128]])
                                nc.vector.tensor_copy(
                                    ktf[h][0:64, 256 * n : 256 * n + 256], src0)
                                nc.scalar.copy(
                                    ktf[h][64:128, 256 * n : 256 * n + 256], src1)
                            elif kind == "q":
                                nc.scalar.copy(
                                    qt2[h][0:64, 512 * n : 512 * n + 512], P[rows, :])
                                # shifted rows: target v where (v+128)%L in window
                                if n == 0:
                                    nc.vector.tensor_copy(
                                        qt2[h][64:128, L - 128 : L], P[rows, 0:128])
                                else:
                                    nc.vector.tensor_copy(
                                        qt2[h][64:128, 512 * n - 128 : 512 * n], P[rows, 0:128])
                                nc.vector.tensor_copy(
                                    qt2[h][64:128, 512 * n : 512 * n + 384], P[rows, 128:512])
                        if kind == "v":
                            nc.scalar.copy(vt[m][:, 512 * n : 512 * n + 512], P[:, :])
                    del xst

            proj_pass("xk", "wk", "k")
            load_w("wq")
            load_w("wv")
            proj_pass("xq", "wq", "q")
            for q in range(2):
                t = wp.tile([128, D_MODEL], f16, tag=f"wo{q}", name=f"wo{q}")
                nc.sync.dma_start(out=t[:, :], in_=ins["wo"][q * 128 : (q + 1) * 128, :])
                wo_t.append(t)
            # qt2 margin cols [4096,4352) = cols [0,256)
            for h in range(HPC):
                nc.vector.tensor_copy(qt2[h][:, L : L + 256], qt2[h][:, 0:256])
            proj_pass("xv", "wv", "v")
            # vext: duplicated V rows
            for q in range(2):
                nc.sync.dma_start(
                    out=AP(vext[q].tensor, 0, [[VEXT, 128], [1, L]]), in_=vt[q][:, :])
                nc.sync.dma_start(
                    out=AP(vext[q].tensor, L, [[VEXT, 128], [1, L]]), in_=vt[q][:, :])

            # ---- racc per head -> rext DRAM
            def racc_head(h):
                for w in range(NW):
                    P = pp.tile([128, 512], f32, tag="mm", bufs=6)
                    for j in range(16):
                        s = (512 * w + 256 * j) % L
                        nc.tensor.matmul(
                            P[:, :],
                            ktf[h][:, 128 * j : 128 * j + 128],
                            qt2[h][:, s : s + 512],
                            start=(j == 0),
                            stop=(j == 15),
                            skip_group_check=True,
                        )
                    rb = wk_.tile([128, 512], f16, tag="rb", bufs=3)
                    nc.scalar.copy(rb[:, :], P[:, :])
                    nc.sync.dma_start(
                        out=AP(rx[h][w // 2].tensor, (512 * w) % 1024,
                               [[1152, 128], [1, 512]]),
                        in_=rb[:, :],
                    )
                    if w >= 2 and w % 2 == 0:
                        nc.sync.dma_start(
                            out=AP(rx[h][w // 2 - 1].tensor, 1024,
                                   [[1152, 128], [1, 128]]),
                            in_=rb[:, 0:128],
                        )
                    if w == 0:
                        nc.sync.dma_start(
                            out=AP(rx[h][3].tensor, 1024,
                                   [[1152, 128], [1, 128]]),
                            in_=rb[:, 0:128],
                        )

            # ---- shear + corr + topk + softmax per head
            tif_t, w8_t = {}, {}

            def corr_head(h):
                shr = wk_.tile([128, L], f16, tag="shr", bufs=2)
                for s4 in range(4):
                    nc.sync.dma_start(
                        out=shr[:, 1024 * s4 : 1024 * (s4 + 1)],
                        in_=AP(rx[h][s4].tensor, 0, [[1153, 128], [1, 1024]]),
                    )
                co = wk_.tile([1, L], f32, tag="co", bufs=1, name=f"co{h}")
                for w in range(NW):
                    P = pp.tile([128, 512], f32, tag="mm", bufs=6)
                    nc.tensor.matmul(
                        P[0:1, :], ones16[:, :], shr[:, 512 * w : 512 * (w + 1)],
                        start=True, stop=True, skip_group_check=True,
                    )
                    nc.scalar.copy(co[:, 512 * w : 512 * (w + 1)], P[0:1, :])

                tv = sm.tile([1, 8], f32, tag=f"tv{h}", name=f"tv{h}")
                ti = sm.tile([1, 8], u32, tag=f"ti{h}", name=f"ti{h}")
                nc.vector.max_with_indices(tv[:, :], ti[:, :], co[:, :])
                tif = sm.tile([1, 8], f32, tag=f"tif{h}", name=f"tif{h}")
                nc.vector.tensor_copy(tif[:, :], ti[:, :])
                tif_t[h] = tif

                negmax = sm.tile([1, 1], f32, tag=f"nm{h}", name=f"nm{h}")
                nc.vector.tensor_scalar_mul(negmax[:, :], tv[:, 0:1], -1.0)
                e8 = sm.tile([1, 8], f32, tag=f"e8{h}", name=f"e8{h}")
                nc.scalar.activation(
                    e8[:, :], tv[:, :], mybir.ActivationFunctionType.Exp,
                    bias=negmax[:, 0:1], scale=1.0,
                )
                ssum = sm.tile([1, 1], f32, tag=f"ss{h}", name=f"ss{h}")
                nc.vector.tensor_reduce(
                    ssum[:, :], e8[:, :], mybir.AxisListType.X, mybir.AluOpType.add
                )
                rs = sm.tile([1, 1], f32, tag=f"rs{h}", name=f"rs{h}")
                nc.vector.reciprocal(rs[:, :], ssum[:, :])
                w8 = sm.tile([1, 8], f32, tag=f"w8{h}", name=f"w8{h}")
                nc.vector.tensor_scalar_mul(w8[:, :], e8[:, :], rs[:, 0:1])
                w8_t[h] = w8

            # ---- weighted circular gather of V per pair
            vw = [None, None]
            vw1b = []  # pair-1 per-block tiles (outproj chases block completion)

            def bcast_pair(q):
                hA, hB = 2 * q, 2 * q + 1
                Pb = pp.tile([128, 16], f32, tag="bc", bufs=1)
                nc.tensor.matmul(Pb[:, 0:8], masks_t[0:1, 0:128], tif_t[hA][:, :],
                                 start=True, stop=False, skip_group_check=True)
                nc.tensor.matmul(Pb[:, 0:8], masks_t[0:1, 128:256], tif_t[hB][:, :],
                                 start=False, stop=False, skip_group_check=True)
                nc.tensor.matmul(Pb[:, 0:8], prow_t[0:1, :], ones8[:, :],
                                 start=False, stop=True, skip_group_check=True)
                nc.tensor.matmul(Pb[:, 8:16], masks_t[0:1, 0:128], w8_t[hA][:, :],
                                 start=True, stop=False, skip_group_check=True)
                nc.tensor.matmul(Pb[:, 8:16], masks_t[0:1, 128:256], w8_t[hB][:, :],
                                 start=False, stop=True, skip_group_check=True)
                idxall = sm.tile([128, 8], u32, tag=f"idxa{q}", name=f"idxa{q}")
                nc.vector.tensor_copy(idxall[:, :], Pb[:, 0:8])
                wcol = sm.tile([128, 8], f32, tag=f"wcol{q}", name=f"wcol{q}")
                nc.vector.tensor_copy(wcol[:, :], Pb[:, 8:16])
                return idxall, wcol

            def combine_full(q, idxall, wcol):
                vwa = wk_.tile([128, L], f16, tag=f"vwa{q}", bufs=1, name=f"vwa{q}")
                for k in range(TOP_K):
                    gk = wk_.tile([128, L], f16, tag="gk", bufs=3)
                    nc.gpsimd.indirect_dma_start(
                        out=gk[:, :],
                        out_offset=None,
                        in_=AP(vext[q].tensor, 0, [[VEXT, 128], [1, L]]),
                        in_offset=bass.IndirectOffsetOnAxis(
                            ap=idxall[:, k : k + 1], axis=1),
                    )
                    if k == 0:
                        nc.vector.tensor_scalar(
                            vwa[:, :], gk[:, :], wcol[:, 0:1], None,
                            mybir.AluOpType.mult,
                        )
                    else:
                        nc.vector.scalar_tensor_tensor(
                            vwa[:, :], gk[:, :], wcol[:, k : k + 1], vwa[:, :],
                            mybir.AluOpType.mult, mybir.AluOpType.add,
                        )
                return vwa

            def combine_block(q, blk, idxall, wcol, c0, BL):
                vwa = wk_.tile([128, BL], f16, tag=f"vwa{q}_{blk}", bufs=1,
                               name=f"vwa{q}_{blk}")
                vwb = wk_.tile([128, BL], f16, tag="vwb", bufs=2,
                               name=f"vwb{q}_{blk}")
                acc = [vwa, vwb]
                for k in range(TOP_K):
                    gk = wk_.tile([128, BL], f16, tag=f"gk{q}_{blk}", bufs=4)
                    nc.gpsimd.indirect_dma_start(
                        out=gk[:, :],
                        out_offset=None,
                        in_=AP(vext[q].tensor, 0, [[VEXT, 128], [1, BL]]),
                        in_offset=bass.IndirectOffsetOnAxis(
                            ap=idxall[:, k : k + 1], axis=1),
                        element_offset=c0,
                    )
                    a = acc[k % 2]
                    if k < 2:
                        nc.vector.tensor_scalar(
                            a[:, :], gk[:, :], wcol[:, k : k + 1], None,
                            mybir.AluOpType.mult,
                        )
                    else:
                        nc.vector.scalar_tensor_tensor(
                            a[:, :], gk[:, :], wcol[:, k : k + 1], a[:, :],
                            mybir.AluOpType.mult, mybir.AluOpType.add,
                        )
                nc.vector.tensor_add(vwa[:, :], vwa[:, :], vwb[:, :])
                return vwa

            def outproj_block(c0, mcount, v0full, v1):
                for mm_ in range(mcount):
                    m = c0 // 128 + mm_
                    for nn in range(2):
                        P = pp.tile([128, 512], f32, tag="mm", bufs=6)
                        nc.tensor.matmul(
                            P[:, :], v0full[:, 128 * m : 128 * (m + 1)],
                            wo_t[0][:, 512 * nn : 512 * (nn + 1)],
                            start=True, stop=False, skip_group_check=True,
                        )
                        nc.tensor.matmul(
                            P[:, :], v1[:, 128 * mm_ : 128 * (mm_ + 1)],
                            wo_t[1][:, 512 * nn : 512 * (nn + 1)],
                            start=False, stop=True, skip_group_check=True,
                        )
                        ot = wk_.tile([128, 512], f16, tag="ot", bufs=6)
                        if nn == 0:
                            nc.vector.tensor_copy(ot[:, :], P[:, :])
                        else:
                            nc.scalar.copy(ot[:, :], P[:, :])
                        for hf in range(2):
                            nc.sync.dma_start(
                                out=out_d[128 * m : 128 * (m + 1),
                                          512 * nn + 256 * hf : 512 * nn + 256 * (hf + 1)],
                                in_=ot[:, 256 * hf : 256 * (hf + 1)],
                            )

            # schedule: racc h0,h1 then skewed corr tails; pair-0 combine runs
            # under racc3; pair-1 combine is blocked with outproj chasing it
            racc_head(0)
            racc_head(1)
            corr_head(0)
            racc_head(2)
            corr_head(1)
            bc0 = bcast_pair(0)
            v0full = combine_full(0, *bc0)
            racc_head(3)
            corr_head(2)
            corr_head(3)
            bc1 = bcast_pair(1)
            blocks = [(0, 512), (512, 1536), (2048, 1536), (3584, 512)]
            for blk, (c0, BL) in enumerate(blocks):
                v1 = combine_block(1, blk, *bc1, c0, BL)
                outproj_block(c0, BL // 128, v0full, v1)

    # scrub path-dependent debug info so the NEFF cache key is directory-independent
    import concourse.mybir as mybir
    for alloc in nc.m.functions[0].allocations:
        if isinstance(alloc, mybir.MemoryLocationSet):
            for ml in alloc.memorylocations:
                try:
                    ml.ant_debug = None
                except Exception:
                    pass
    nc.finalize()
    return nc


_build_nc = _fixed_filename(_build_nc_impl)


def _build_into_impl(box):
    box["nc"] = _build_nc()


_build_into = _fixed_filename(_build_into_impl)


def _get_nc():
    if "nc" not in _NC_CACHE:
        # build in a fresh thread: instruction debug tracebacks then capture a
        # deterministic stack (thread bootstrap only), keeping the serialized
        # BIR — and hence the NEFF compile-cache key — caller-independent
        import threading

        box = {}
        t = threading.Thread(target=_build_into, args=(box,), name="acb")
        t.start()
        t.join()
        _NC_CACHE["nc"] = box["nc"]
    return _NC_CACHE["nc"]


def _core_inputs(query, key, value, Wq, Wk, Wv, Wo):
    f16 = np.float16
    masks = np.zeros((1, 256), np.float32)
    masks[0, 0:64] = 1.0
    masks[0, 192:256] = 1.0
    prow = (np.arange(128, dtype=np.float32) * VEXT).reshape(1, 128)
    xT = {}
    for b in range(B):
        xT[("q", b)] = np.ascontiguousarray(query[b].T, dtype=f16)
        xT[("k", b)] = np.ascontiguousarray(key[b].T, dtype=f16)
        xT[("v", b)] = np.ascontiguousarray(value[b].T, dtype=f16)
    in_maps = []
    for c in range(N_CORES):
        b = c // 4
        h0 = (c % 4) * HPC
        cols = slice(h0 * DK, h0 * DK + COLS)
        in_maps.append({
            "xq": xT[("q", b)],
            "xk": xT[("k", b)],
            "xv": xT[("v", b)],
            "wq": Wq[:, cols].astype(f16),
            "wk": (Wk[:, cols] / DK).astype(f16),  # fold corr mean(1/Dk) into K
            "wv": Wv[:, cols].astype(f16),
            "wo": Wo[h0 * DK : h0 * DK + COLS, :].astype(f16),
            "masks": masks,
            "prow": prow,
        })
    return in_maps


class _Runner:
    """Cached AOT-compiled SPMD dispatch with device-side donated zero buffers."""

    def __init__(self, nc):
        import jax
        import jax.numpy as jnp
        from jax.sharding import Mesh, NamedSharding, PartitionSpec
        from jax.experimental.shard_map import shard_map
        import concourse.mybir as mybir
        from concourse.bass2jax import (
            _bass_exec_p, install_neuronx_cc_hook, partition_id_tensor,
        )

        install_neuronx_cc_hook()
        self.jax = jax
        pname = nc.partition_id_tensor.name if nc.partition_id_tensor else None
        in_names, out_names, out_avals = [], [], []
        for alloc in nc.m.functions[0].allocations:
            if not isinstance(alloc, mybir.MemoryLocationSet):
                continue
            name = alloc.memorylocations[0].name
            if alloc.kind == "ExternalInput":
                if name != pname:
                    in_names.append(name)
            elif alloc.kind == "ExternalOutput":
                out_names.append(name)
                out_avals.append(
                    jax.core.ShapedArray(tuple(alloc.tensor_shape), mybir.dt.np(alloc.dtype))
                )
        self.in_names, self.out_names, self.out_avals = in_names, out_names, out_avals
        n_params, n_outs = len(in_names), len(out_avals)
        all_names = in_names + out_names + ([pname] if pname else [])

        def _body(*a):
            operands = list(a)
            if pname is not None:
                operands.append(partition_id_tensor())
            return tuple(
                _bass_exec_p.bind(
                    *operands, out_avals=tuple(out_avals), in_names=tuple(all_names),
                    out_names=tuple(out_names), lowering_input_output_aliases=(),
                    sim_require_finite=True, sim_require_nnan=True, nc=nc,
                )
            )

        devices = jax.devices()[:N_CORES]
        self.mesh = Mesh(np.asarray(devices), ("core",))
        spec = PartitionSpec("core")
        self.sharding = NamedSharding(self.mesh, spec)
        donate = tuple(range(n_params, n_params + n_outs))
        fn = jax.jit(
            shard_map(
                _body, mesh=self.mesh, in_specs=(spec,) * (n_params + n_outs),
                out_specs=(spec,) * n_outs, check_rep=False,
            ),
            donate_argnums=donate, keep_unused=True,
        )
        gin = [
            jax.ShapeDtypeStruct((N_CORES * s[0], *s[1:]), d)
            for s, d in self._global_shapes(nc)
        ]
        gzero = [
            jax.ShapeDtypeStruct((N_CORES * a.shape[0], *a.shape[1:]), a.dtype)
            for a in out_avals
        ]
        self.compiled = fn.lower(*gin, *gzero).compile()
        zshapes = [((N_CORES * a.shape[0], *a.shape[1:]), a.dtype) for a in out_avals]
        self.zeros_fn = jax.jit(
            lambda: tuple(jnp.zeros(s, d) for s, d in zshapes),
            out_shardings=(self.sharding,) * n_outs,
        )

    def _global_shapes(self, nc):
        import concourse.mybir as mybir

        shapes = []
        for alloc in nc.m.functions[0].allocations:
            if not isinstance(alloc, mybir.MemoryLocationSet):
                continue
            name = alloc.memorylocations[0].name
            if alloc.kind == "ExternalInput" and name in self.in_names:
                shapes.append((tuple(alloc.tensor_shape), mybir.dt.np(alloc.dtype)))
        return shapes

    def stage(self, in_maps):
        cat = [
            np.concatenate([np.asarray(m[n]) for m in in_maps], axis=0)
            for n in self.in_names
        ]
        bufs = [self.jax.device_put(a, self.sharding) for a in cat]
        self.jax.block_until_ready(bufs)
        return bufs

    def make_zeros(self):
        z = self.zeros_fn()
        self.jax.block_until_ready(z)
        return z

    def exec_staged(self, bufs, zeros=None):
        if zeros is None:
            zeros = self.make_zeros()
        outs = self.compiled(*bufs, *zeros)
        self.jax.block_until_ready(outs)
        return outs

    def run(self, in_maps):
        outs = self.exec_staged(self.stage(in_maps))
        res = []
        for c in range(N_CORES):
            m = {}
            for i, nm in enumerate(self.out_names):
                s = self.out_avals[i].shape
                m[nm] = np.asarray(outs[i][c * s[0] : (c + 1) * s[0]])
            res.append(m)
        return res


def _get_runner():
    if "runner" not in _NC_CACHE:
        _NC_CACHE["runner"] = _Runner(_get_nc())
    return _NC_CACHE["runner"]


def _forward_device(query, key, value, Wq, bq, Wk, bk, Wv, bv, Wo, bo):
    runner = _get_runner()
    in_maps = _core_inputs(query, key, value, Wq, Wk, Wv, Wo)
    results = runner.run(in_maps)

    out = np.zeros((B, L, D_MODEL), dtype=np.float32)
    for c in range(N_CORES):
        out[c // 4] += np.asarray(results[c]["out"], dtype=np.float32)
    out += bo.astype(np.float32)
    return out


def kernel(**inputs):
    inputs = {k: np.asarray(v, dtype=np.float32) for k, v in inputs.items()}
    if any(np.any(inputs[k]) for k in ("bq", "bk", "bv")):
        return _forward_host(**inputs)
    try:
        return _forward_device(**inputs)
    except Exception:
        import traceback

        traceback.print_exc()
        return _forward_host(**inputs)
